# revision 20
# baseline (speedup 1.0000x reference)
"""Trainium2 Bass kernel for nn_ApproxExp_FXP32in16out14 (histogram_binning).

Reference semantics: fixed-point piecewise-linear LUT approximation of exp(x)
over 17 uniform breakpoints on [-10, 4] (FXP32.16 in, FXP16.14 out), including
int32-wraparound artifacts of the torch reference in segments 14/15.

The LUT values y0[k] = rint(2^14 exp(-10+0.875k)) are geometric to ~0.35% for
the segments that contain data, and the interpolation weight is affine in x, so
the whole map factors as

    out(x) ~= exp(0.875*k - c0) * ((8/7)*x - k + c1),   k = rne((8/7)*x + 153/14)

The host feeds x' = (8/7)*x + c1' as fp16 (halving input HBM traffic), so the
device pipeline is
    kq = rne(x' + qbias)     int16 (RNE via dtype convert; split between
                             ScalarE Copy-activation and GpSimd tensor_scalar
                             to balance engine load)
    ys = exp(0.875*kq + b')  fp16, ScalarE Exp (k recentered by -11 so fp16
                             intermediates stay near 1.0)
    vt = x' - kq             fp16, DVE tensor_tensor (all-16-bit -> 2x)
    ot = vt * ys             fp16, DVE tensor_tensor (all-16-bit -> 2x)
and the fp16 output is upcast to fp32 on the host. A deterministic ~0.3% of
elements (the int32-wraparound bands at x>=2.7773, the x>=4 clamp, deep tail
x<-4.7) is recomputed exactly on host, from the original fp32 x.

DMA layout (per core, 32 DMA tiles of [128, 8192] fp16, compute on 4096
halves): input DMAs on the sync HWDGE ring, output DMAs on the scalar HWDGE
ring (dispatched ~1.5 tiles late so the ACT stream never stalls on them) —
two independent FIFO rings, 16 KiB/partition descriptors on both streams,
no SWDGE descriptor-ring contention. Per-slot DMA semaphores keep completion
ordering sound.

Sharding: pure data parallel, leading dim 64 -> 8 cores x 8.
"""

import math
from contextlib import ExitStack

import numpy as np

import concourse.bass as bass
import concourse.mybir as mybir
from concourse.bass_utils import run_bass_kernel_spmd

# ---------------------------------------------------------------- constants
FULL_SHAPE = (64, 4096, 1024)
N_CORES = 8
DT, P, FD = 32, 128, 8192  # per-core: 32 DMA tiles of [128, 8192]
FC = FD // 2               # compute half-tile free dim
H = 2 * DT                 # number of compute half-tiles

N_SL = 4   # DMA slot ring depth (in and out)
N_C = 3    # compute ring depth (kq / ys / vt), in half-tiles

# k is shifted down by an integer constant so the DVE intermediate
# vt = x' - (k-11) stays in [0.7, 1.7] where fp16 has ~2^-11 ulp
# (integer shifts commute with RNE quantization, so semantics are unchanged).
KQ_SHIFT = 11

RHO = math.exp(0.875) - 1.0
CONST = 1.0 + RHO / 32768.0          # +0.5 LSB rounding offset of t_fx in Q14
B_SL = RHO / CONST                   # k-coefficient before unit-rescale
AK_SCALE = 8.0 / 7.0                 # 65536/57344
AK_BIAS = 153.0 / 14.0               # 655360/57344 - 0.5
A2_SCALE = 0.875
A2_BIAS = -10.0 + math.log(CONST) + math.log(B_SL)
CONST1 = 1.0 + (655360.0 / 57344.0) * RHO / CONST
T3_ADD = CONST1 / B_SL
# shifted-k variants
AK_BIAS_S = AK_BIAS - KQ_SHIFT
A2_BIAS_S = A2_BIAS + A2_SCALE * KQ_SHIFT
T3_ADD_S = T3_ADD - KQ_SHIFT         # host prescale offset: x' = (8/7)x + T3_ADD_S
QBIAS = AK_BIAS_S - T3_ADD_S         # quantizer bias applied to x'

# host-fixup region boundaries (float32 compares on raw x)
FIX_HI = np.float32(2.7773)          # below first int32-wrap threshold (2.77735)
FIX_LO = np.float32(-4.7)            # deep tail: LUT quantization breaks the model

# ------------------------------------------------------------ bass builder
_NC = None


def _quant_on_act(h: int) -> bool:
    # every 4th half-tile quantizes on ScalarE, the rest on GpSimd — balances
    # ACT (Exp-heavy) against GpSimd (slower per element) at ~300us each
    return h % 4 == 0


def _build_nc() -> bass.Bass:
    global _NC
    if _NC is not None:
        return _NC
    f32, f16, i16 = mybir.dt.float32, mybir.dt.float16, mybir.dt.int16
    nc = bass.Bass()
    x_ext = nc.declare_dram_parameter("x", [DT, P, FD], f16, isOutput=False)
    o_ext = nc.declare_dram_parameter("out", [DT, P, FD], f16, isOutput=True)

    # [128,1] constant for the Exp activation bias (const_aps only has 0/1).
    bias_t = nc.alloc_sbuf_tensor("const-a2bias", [P, 1], f32)
    a2_bias_ap = bias_t.ap()

    ctx = ExitStack()
    # One backing tensor per DMA ring so each 8192-wide DMA slot is two
    # contiguous 4096-wide compute halves.
    xt = ctx.enter_context(nc.sbuf_tensor("xt", [P, N_SL * FD], f16))
    ot = ctx.enter_context(nc.sbuf_tensor("ot", [P, N_SL * FD], f16))
    kq = [ctx.enter_context(nc.sbuf_tensor(f"kq{j}", [P, FC], i16)) for j in range(N_C)]
    ys = [ctx.enter_context(nc.sbuf_tensor(f"ys{j}", [P, FC], f16)) for j in range(N_C)]
    vt = [ctx.enter_context(nc.sbuf_tensor(f"vt{j}", [P, FC], f16)) for j in range(N_C)]
    s_in = [ctx.enter_context(nc.semaphore(f"s_in{j}")) for j in range(N_SL)]
    s_out = [ctx.enter_context(nc.semaphore(f"s_out{j}")) for j in range(N_SL)]
    s_ka = ctx.enter_context(nc.semaphore("s_ka"))  # ACT-produced quants
    s_kg = ctx.enter_context(nc.semaphore("s_kg"))  # GpSimd-produced quants
    s_y = ctx.enter_context(nc.semaphore("s_y"))
    s_v1 = ctx.enter_context(nc.semaphore("s_v1"))
    s_o = ctx.enter_context(nc.semaphore("s_o"))
    s_bias = ctx.enter_context(nc.semaphore("s_bias"))
    block = ctx.enter_context(nc.Block())

    # cumulative quant counts: after quant(h), s_ka == NA[h], s_kg == NG[h]
    NA, NG = [], []
    na = ng = 0
    for h in range(H):
        if _quant_on_act(h):
            na += 1
        else:
            ng += 1
        NA.append(na)
        NG.append(ng)

    def wait_quant(engine, h):
        """Wait until quant(h) is complete (from either producer)."""
        if _quant_on_act(h):
            engine.wait_ge(s_ka, NA[h])
        else:
            engine.wait_ge(s_kg, NG[h])

    def xh(h):  # compute half h of the input ring
        t = (h // 2) % N_SL
        return xt[:, t * FD + (h % 2) * FC : t * FD + (h % 2) * FC + FC]

    def oh(h):  # compute half h of the output ring
        t = (h // 2) % N_SL
        return ot[:, t * FD + (h % 2) * FC : t * FD + (h % 2) * FC + FC]

    def in_wait(engine, h):
        t = h // 2
        engine.wait_ge(s_in[t % N_SL], 16 * (t // N_SL + 1))

    @block.sync
    def _(sync):
        for t in range(DT):
            if t >= N_SL:
                # slot (t-N_SL) free once both halves went through DVE T2
                # (T2 consumed xt and kq, so the quant read is implied)
                sync.wait_ge(s_v1, 2 * (t - N_SL) + 2)
            sync.dma_start(
                out=xt[:, (t % N_SL) * FD : (t % N_SL + 1) * FD], in_=x_ext[t]
            ).then_inc(s_in[t % N_SL], 16)

    @block.gpsimd
    def _(gpsimd):
        nc.gpsimd.memset(a2_bias_ap, A2_BIAS_S).then_inc(s_bias, 1)
        for h in range(H):
            if _quant_on_act(h):
                continue
            in_wait(gpsimd, h)
            if h >= N_C:
                # kq slot free: T2(h-N_C) and Exp(h-N_C) both read it
                gpsimd.wait_ge(s_v1, h - N_C + 1)
                gpsimd.wait_ge(s_y, h - N_C + 1)
            nc.gpsimd.tensor_scalar(
                out=kq[h % N_C][:], in0=xh(h), scalar1=QBIAS, scalar2=None,
                op0=mybir.AluOpType.add,
            ).then_inc(s_kg, 1)

    @block.scalar
    def _(scalar):
        scalar.wait_ge(s_bias, 1)
        for h in range(H):
            t = h // 2
            if _quant_on_act(h):
                in_wait(scalar, h)
                if h >= N_C:
                    scalar.wait_ge(s_v1, h - N_C + 1)  # kq slot: T2(h-N_C)
                nc.scalar.activation(
                    kq[h % N_C][:], xh(h), mybir.ActivationFunctionType.Copy,
                    bias=QBIAS, scale=1.0,
                ).then_inc(s_ka, 1)
            if h >= N_C:
                scalar.wait_ge(s_o, h - N_C + 1)  # ys slot: T3(h-N_C) done
            wait_quant(scalar, h)  # covers own-engine ACT pipeline RAW too
            nc.scalar.activation(
                ys[h % N_C][:], kq[h % N_C][:], mybir.ActivationFunctionType.Exp,
                bias=a2_bias_ap, scale=A2_SCALE,
            ).then_inc(s_y, 1)
            # Output DMA for tile (h-3)//2, dispatched well after its T3 so
            # this wait is almost always already satisfied.
            if h % 2 == 1 and h >= 3:
                to = (h - 3) // 2
                scalar.wait_ge(s_o, 2 * to + 2)
                nc.scalar.dma_start(
                    out=o_ext[to], in_=ot[:, (to % N_SL) * FD : (to % N_SL + 1) * FD]
                ).then_inc(s_out[to % N_SL], 16)
        for to in range(DT - 1, DT):  # flush the last tile
            scalar.wait_ge(s_o, 2 * to + 2)
            nc.scalar.dma_start(
                out=o_ext[to], in_=ot[:, (to % N_SL) * FD : (to % N_SL + 1) * FD]
            ).then_inc(s_out[to % N_SL], 16)

    @block.vector
    def _(vector):
        for h in range(H):
            t = h // 2
            in_wait(vector, h)
            wait_quant(vector, h)
            # T2: vt = x' - kq   (fp16/int16 -> 2x)
            nc.vector.tensor_tensor(
                out=vt[h % N_C][:], in0=xh(h), in1=kq[h % N_C][:],
                op=mybir.AluOpType.subtract,
            ).then_inc(s_v1, 1)
            vector.wait_ge(s_y, h + 1)
            vector.wait_ge(s_v1, h + 1)  # own-engine RAW on vt (pipeline drain)
            if t >= N_SL and h % 2 == 0:
                vector.wait_ge(s_out[t % N_SL], 16 * (t // N_SL))  # slot drained
            # T3: ot = vt * ys   (all operands fp16 -> 2x)
            nc.vector.tensor_tensor(
                out=oh(h), in0=vt[h % N_C][:], in1=ys[h % N_C][:],
                op=mybir.AluOpType.mult,
            ).then_inc(s_o, 1)

    ctx.close()
    _NC = nc
    return nc


# ------------------------------------------------- exact host-side reference
_XP = np.round(np.linspace(-10.0, 4.0, 17) * 65536.0).astype(np.int64)
_YV = np.round(np.exp(np.linspace(-10.0, 4.0, 17)) * 16384.0).astype(np.int64)
_DY = np.diff(_YV)


def _reference_exact(xs: np.ndarray) -> np.ndarray:
    """Bit-faithful int32 reference for a (small) subset of elements."""
    x_int = np.rint(xs.astype(np.float64) * 65536.0).astype(np.int64)
    mask_low = x_int <= _XP[0]
    mask_high = x_int >= _XP[-1]
    xc = np.clip(x_int, _XP[0], _XP[-1])
    idx = np.clip(np.searchsorted(_XP, xc, side="left") - 1, 0, 15)
    dxv = xc - _XP[idx]
    t_fx = ((dxv << 14) + 28672) // 57344
    prod = t_fx * _DY[idx] + 8192
    pm = prod & 0xFFFFFFFF
    S = np.where(pm >= 1 << 31, pm - (1 << 32), pm)
    interp = _YV[idx] + (S >> 14)
    out_int = np.where(mask_low, _YV[0], np.where(mask_high, _YV[-1], interp))
    return (out_int.astype(np.float32) / np.float32(16384.0)).astype(np.float32)


def _host_fixup(x_flat: np.ndarray, out_flat: np.ndarray) -> None:
    sel = (x_flat >= FIX_HI) | (x_flat < FIX_LO)
    idxs = np.flatnonzero(sel)
    if idxs.size:
        out_flat[idxs] = _reference_exact(x_flat[idxs])


_last_results = None


def kernel(x: np.ndarray) -> np.ndarray:
    assert x.shape == FULL_SHAPE and x.dtype == np.float32, (x.shape, x.dtype)
    nc = _build_nc()
    per = FULL_SHAPE[0] // N_CORES
    # host prescale to fp16
    xp = (np.float32(AK_SCALE) * x + np.float32(T3_ADD_S)).astype(np.float16)
    in_maps = [
        {"x": np.ascontiguousarray(xp[i * per : (i + 1) * per]).reshape(DT, P, FD)}
        for i in range(N_CORES)
    ]
    global _last_results
    res = run_bass_kernel_spmd(nc, in_maps, core_ids=list(range(N_CORES)))
    _last_results = res
    out = np.concatenate(
        [
            r["out"].astype(np.float32).reshape(per, FULL_SHAPE[1], FULL_SHAPE[2])
            for r in res.results
        ],
        axis=0,
    )
    _host_fixup(x.ravel(), out.ravel())
    return out


# revision 25
# speedup vs baseline: 6.1561x; 6.1561x over previous
"""Trainium2 Bass kernel for nn_ApproxExp_FXP32in16out14 (histogram_binning).

Reference semantics: fixed-point piecewise-linear LUT approximation of exp(x)
over 17 uniform breakpoints on [-10, 4] (FXP32.16 in, FXP16.14 out), including
int32-wraparound artifacts of the torch reference in segments 14/15.

The LUT values y0[k] = rint(2^14 exp(-10+0.875k)) are geometric to ~0.35% for
the segments that contain data, and the interpolation weight is affine in x, so
the whole map factors as

    out(x) ~= exp(0.875*k - c0) * ((8/7)*x - k + c1),   k = rne((8/7)*x + 153/14)

The host feeds x' = (8/7)*x + c1' as fp16 (halving input HBM traffic), so the
device pipeline is
    kq = rne(x' + qbias)     int16 (RNE via dtype convert; split between
                             ScalarE Copy-activation and GpSimd tensor_scalar
                             to balance engine load)
    ys = exp(0.875*kq + b')  fp16, ScalarE Exp (k recentered by -11 so fp16
                             intermediates stay near 1.0)
    vt = x' - kq             fp16, DVE tensor_tensor (all-16-bit -> 2x)
    ot = vt * ys             fp16, DVE tensor_tensor (all-16-bit -> 2x)
and the fp16 output is upcast to fp32 on the host. A deterministic ~0.3% of
elements (the int32-wraparound bands at x>=2.7773, the x>=4 clamp, deep tail
x<-4.7) is recomputed exactly on host, from the original fp32 x.

DMA layout (per core, 32 DMA tiles of [128, 8192] fp16, compute on 4096
halves): input DMAs on the sync HWDGE ring, output DMAs on the scalar HWDGE
ring (dispatched ~1.5 tiles late so the ACT stream never stalls on them) —
two independent FIFO rings, 16 KiB/partition descriptors on both streams,
no SWDGE descriptor-ring contention. Per-slot DMA semaphores keep completion
ordering sound.

Sharding: pure data parallel, leading dim 64 -> 8 cores x 8.
"""

import math
from contextlib import ExitStack

import numpy as np

import concourse.bass as bass
import concourse.mybir as mybir
from concourse.bass_utils import run_bass_kernel_spmd

# ---------------------------------------------------------------- constants
FULL_SHAPE = (64, 4096, 1024)
N_CORES = 8
DT, P, FD = 32, 128, 8192  # per-core: 32 DMA tiles of [128, 8192]
FC = FD // 2               # compute half-tile free dim
H = 2 * DT                 # number of compute half-tiles

N_SL = 4   # DMA slot ring depth (in and out)
N_C = 3    # compute ring depth (kq / ys / vt), in half-tiles

# k is shifted down by an integer constant so the DVE intermediate
# vt = x' - (k-11) stays in [0.7, 1.7] where fp16 has ~2^-11 ulp
# (integer shifts commute with RNE quantization, so semantics are unchanged).
KQ_SHIFT = 11

RHO = math.exp(0.875) - 1.0
CONST = 1.0 + RHO / 32768.0          # +0.5 LSB rounding offset of t_fx in Q14
B_SL = RHO / CONST                   # k-coefficient before unit-rescale
AK_SCALE = 8.0 / 7.0                 # 65536/57344
AK_BIAS = 153.0 / 14.0               # 655360/57344 - 0.5
A2_SCALE = 0.875
A2_BIAS = -10.0 + math.log(CONST) + math.log(B_SL)
CONST1 = 1.0 + (655360.0 / 57344.0) * RHO / CONST
T3_ADD = CONST1 / B_SL
# shifted-k variants
AK_BIAS_S = AK_BIAS - KQ_SHIFT
A2_BIAS_S = A2_BIAS + A2_SCALE * KQ_SHIFT
T3_ADD_S = T3_ADD - KQ_SHIFT         # host prescale offset: x' = (8/7)x + T3_ADD_S
QBIAS = AK_BIAS_S - T3_ADD_S         # quantizer bias applied to x'

# host-fixup region boundaries (float32 compares on raw x)
FIX_HI = np.float32(2.7773)          # below first int32-wrap threshold (2.77735)
FIX_LO = np.float32(-4.7)            # deep tail: LUT quantization breaks the model

# ------------------------------------------------------------ bass builder
_NC = None


def _quant_on_act(h: int) -> bool:
    # 5/12 of half-tiles quantize on ScalarE (Copy activation, 3.7us), the
    # rest on DVE (tensor_scalar, ~1.3us) — balances both engines at ~340us
    return h % 12 >= 7


def _build_nc() -> bass.Bass:
    global _NC
    if _NC is not None:
        return _NC
    f32, f16, i16 = mybir.dt.float32, mybir.dt.float16, mybir.dt.int16
    nc = bass.Bass()
    x_ext = nc.declare_dram_parameter("x", [DT, P, FD], f16, isOutput=False)
    o_ext = nc.declare_dram_parameter("out", [DT, P, FD], f16, isOutput=True)

    # [128,1] constant for the Exp activation bias (const_aps only has 0/1).
    bias_t = nc.alloc_sbuf_tensor("const-a2bias", [P, 1], f32)
    a2_bias_ap = bias_t.ap()

    ctx = ExitStack()
    # One backing tensor per DMA ring so each 8192-wide DMA slot is two
    # contiguous 4096-wide compute halves.
    xt = ctx.enter_context(nc.sbuf_tensor("xt", [P, N_SL * FD], f16))
    ot = ctx.enter_context(nc.sbuf_tensor("ot", [P, N_SL * FD], f16))
    kq = [ctx.enter_context(nc.sbuf_tensor(f"kq{j}", [P, FC], i16)) for j in range(N_C)]
    ys = [ctx.enter_context(nc.sbuf_tensor(f"ys{j}", [P, FC], f16)) for j in range(N_C)]
    vt = [ctx.enter_context(nc.sbuf_tensor(f"vt{j}", [P, FC], f16)) for j in range(N_C)]
    s_in = [ctx.enter_context(nc.semaphore(f"s_in{j}")) for j in range(N_SL)]
    s_out = [ctx.enter_context(nc.semaphore(f"s_out{j}")) for j in range(N_SL)]
    s_ka = ctx.enter_context(nc.semaphore("s_ka"))  # ACT-produced quants
    s_kd = ctx.enter_context(nc.semaphore("s_kd"))  # DVE-produced quants
    s_y = ctx.enter_context(nc.semaphore("s_y"))
    s_v1 = ctx.enter_context(nc.semaphore("s_v1"))
    s_o = ctx.enter_context(nc.semaphore("s_o"))
    s_bias = ctx.enter_context(nc.semaphore("s_bias"))
    block = ctx.enter_context(nc.Block())

    # cumulative quant counts: after quant(h), s_ka == NA[h], s_kg == NG[h]
    NA, NG = [], []
    na = ng = 0
    for h in range(H):
        if _quant_on_act(h):
            na += 1
        else:
            ng += 1
        NA.append(na)
        NG.append(ng)

    def wait_quant(engine, h):
        """Wait until quant(h) is complete (from either producer)."""
        if _quant_on_act(h):
            engine.wait_ge(s_ka, NA[h])
        else:
            engine.wait_ge(s_kd, NG[h])

    def xh(h):  # compute half h of the input ring
        t = (h // 2) % N_SL
        return xt[:, t * FD + (h % 2) * FC : t * FD + (h % 2) * FC + FC]

    def oh(h):  # compute half h of the output ring
        t = (h // 2) % N_SL
        return ot[:, t * FD + (h % 2) * FC : t * FD + (h % 2) * FC + FC]

    def in_wait(engine, h):
        t = h // 2
        engine.wait_ge(s_in[t % N_SL], 16 * (t // N_SL + 1))

    @block.sync
    def _(sync):
        for t in range(DT):
            if t >= N_SL:
                # slot (t-N_SL) free once both halves went through DVE T2
                # (T2 consumed xt and kq, so the quant read is implied)
                sync.wait_ge(s_v1, 2 * (t - N_SL) + 2)
            sync.dma_start(
                out=xt[:, (t % N_SL) * FD : (t % N_SL + 1) * FD], in_=x_ext[t]
            ).then_inc(s_in[t % N_SL], 16)

    @block.gpsimd
    def _(gpsimd):
        nc.gpsimd.memset(a2_bias_ap, A2_BIAS_S).then_inc(s_bias, 1)

    @block.scalar
    def _(scalar):
        scalar.wait_ge(s_bias, 1)
        for h in range(H):
            t = h // 2
            if _quant_on_act(h):
                in_wait(scalar, h)
                if h >= N_C:
                    scalar.wait_ge(s_v1, h - N_C + 1)  # kq slot: T2(h-N_C)
                nc.scalar.activation(
                    kq[h % N_C][:], xh(h), mybir.ActivationFunctionType.Copy,
                    bias=QBIAS, scale=1.0,
                ).then_inc(s_ka, 1)
            if h >= N_C:
                scalar.wait_ge(s_o, h - N_C + 1)  # ys slot: T3(h-N_C) done
            wait_quant(scalar, h)  # covers own-engine ACT pipeline RAW too
            nc.scalar.activation(
                ys[h % N_C][:], kq[h % N_C][:], mybir.ActivationFunctionType.Exp,
                bias=a2_bias_ap, scale=A2_SCALE,
            ).then_inc(s_y, 1)
            # Output DMA for tile (h-3)//2, dispatched well after its T3 so
            # this wait is almost always already satisfied.
            if h % 2 == 1 and h >= 3:
                to = (h - 3) // 2
                scalar.wait_ge(s_o, 2 * to + 2)
                nc.scalar.dma_start(
                    out=o_ext[to], in_=ot[:, (to % N_SL) * FD : (to % N_SL + 1) * FD]
                ).then_inc(s_out[to % N_SL], 16)
        for to in range(DT - 1, DT):  # flush the last tile
            scalar.wait_ge(s_o, 2 * to + 2)
            nc.scalar.dma_start(
                out=o_ext[to], in_=ot[:, (to % N_SL) * FD : (to % N_SL + 1) * FD]
            ).then_inc(s_out[to % N_SL], 16)

    @block.vector
    def _(vector):
        for h in range(H):
            t = h // 2
            in_wait(vector, h)
            if _quant_on_act(h):
                wait_quant(vector, h)
            else:
                if h >= N_C:
                    vector.wait_ge(s_y, h - N_C + 1)  # kq slot: Exp(h-N_C) done
                nc.vector.tensor_scalar(
                    out=kq[h % N_C][:], in0=xh(h), scalar1=QBIAS, scalar2=None,
                    op0=mybir.AluOpType.add,
                ).then_inc(s_kd, 1)
                vector.wait_ge(s_kd, NG[h])  # own-engine RAW on kq
            # T2: vt = x' - kq   (fp16/int16 -> 2x)
            nc.vector.tensor_tensor(
                out=vt[h % N_C][:], in0=xh(h), in1=kq[h % N_C][:],
                op=mybir.AluOpType.subtract,
            ).then_inc(s_v1, 1)
            vector.wait_ge(s_y, h + 1)
            vector.wait_ge(s_v1, h + 1)  # own-engine RAW on vt (pipeline drain)
            if t >= N_SL and h % 2 == 0:
                vector.wait_ge(s_out[t % N_SL], 16 * (t // N_SL))  # slot drained
            # T3: ot = vt * ys   (all operands fp16 -> 2x)
            nc.vector.tensor_tensor(
                out=oh(h), in0=vt[h % N_C][:], in1=ys[h % N_C][:],
                op=mybir.AluOpType.mult,
            ).then_inc(s_o, 1)

    ctx.close()
    _NC = nc
    return nc


# ------------------------------------------------- exact host-side reference
_XP = np.round(np.linspace(-10.0, 4.0, 17) * 65536.0).astype(np.int64)
_YV = np.round(np.exp(np.linspace(-10.0, 4.0, 17)) * 16384.0).astype(np.int64)
_DY = np.diff(_YV)


def _reference_exact(xs: np.ndarray) -> np.ndarray:
    """Bit-faithful int32 reference for a (small) subset of elements."""
    x_int = np.rint(xs.astype(np.float64) * 65536.0).astype(np.int64)
    mask_low = x_int <= _XP[0]
    mask_high = x_int >= _XP[-1]
    xc = np.clip(x_int, _XP[0], _XP[-1])
    idx = np.clip(np.searchsorted(_XP, xc, side="left") - 1, 0, 15)
    dxv = xc - _XP[idx]
    t_fx = ((dxv << 14) + 28672) // 57344
    prod = t_fx * _DY[idx] + 8192
    pm = prod & 0xFFFFFFFF
    S = np.where(pm >= 1 << 31, pm - (1 << 32), pm)
    interp = _YV[idx] + (S >> 14)
    out_int = np.where(mask_low, _YV[0], np.where(mask_high, _YV[-1], interp))
    return (out_int.astype(np.float32) / np.float32(16384.0)).astype(np.float32)


def _host_fixup(x_flat: np.ndarray, out_flat: np.ndarray) -> None:
    sel = (x_flat >= FIX_HI) | (x_flat < FIX_LO)
    idxs = np.flatnonzero(sel)
    if idxs.size:
        out_flat[idxs] = _reference_exact(x_flat[idxs])


_last_results = None


def kernel(x: np.ndarray) -> np.ndarray:
    assert x.shape == FULL_SHAPE and x.dtype == np.float32, (x.shape, x.dtype)
    nc = _build_nc()
    per = FULL_SHAPE[0] // N_CORES
    # host prescale to fp16
    xp = (np.float32(AK_SCALE) * x + np.float32(T3_ADD_S)).astype(np.float16)
    in_maps = [
        {"x": np.ascontiguousarray(xp[i * per : (i + 1) * per]).reshape(DT, P, FD)}
        for i in range(N_CORES)
    ]
    global _last_results
    res = run_bass_kernel_spmd(nc, in_maps, core_ids=list(range(N_CORES)))
    _last_results = res
    out = np.concatenate(
        [
            r["out"].astype(np.float32).reshape(per, FULL_SHAPE[1], FULL_SHAPE[2])
            for r in res.results
        ],
        axis=0,
    )
    _host_fixup(x.ravel(), out.ravel())
    return out


# revision 27
# speedup vs baseline: 7.0639x; 1.1475x over previous
"""Trainium2 Bass kernel for nn_ApproxExp_FXP32in16out14 (histogram_binning).

Reference semantics: fixed-point piecewise-linear LUT approximation of exp(x)
over 17 uniform breakpoints on [-10, 4] (FXP32.16 in, FXP16.14 out), including
int32-wraparound artifacts of the torch reference in segments 14/15.

The LUT values y0[k] = rint(2^14 exp(-10+0.875k)) are geometric to ~0.35% for
the segments that contain data, and the interpolation weight is affine in x, so
the whole map factors as

    out(x) ~= exp(0.875*k - c0) * ((8/7)*x - k + c1),   k = rne((8/7)*x + 153/14)

The host feeds x' = (8/7)*x + c1' as fp16 (halving input HBM traffic), so the
device pipeline is
    kq = rne(x' + qbias)     int16 (RNE via dtype convert; split between
                             ScalarE Copy-activation and GpSimd tensor_scalar
                             to balance engine load)
    ys = exp(0.875*kq + b')  fp16, ScalarE Exp (k recentered by -11 so fp16
                             intermediates stay near 1.0)
    vt = x' - kq             fp16, DVE tensor_tensor (all-16-bit -> 2x)
    ot = vt * ys             fp16, DVE tensor_tensor (all-16-bit -> 2x)
and the fp16 output is upcast to fp32 on the host. A deterministic ~0.3% of
elements (the int32-wraparound bands at x>=2.7773, the x>=4 clamp, deep tail
x<-4.7) is recomputed exactly on host, from the original fp32 x.

DMA layout (per core, 32 DMA tiles of [128, 8192] fp16, compute on 4096
halves): input DMAs on the sync HWDGE ring, output DMAs on the scalar HWDGE
ring (dispatched ~1.5 tiles late so the ACT stream never stalls on them) —
two independent FIFO rings, 16 KiB/partition descriptors on both streams,
no SWDGE descriptor-ring contention. Per-slot DMA semaphores keep completion
ordering sound.

Sharding: pure data parallel, leading dim 64 -> 8 cores x 8.
"""

import math
from contextlib import ExitStack

import numpy as np

import concourse.bass as bass
import concourse.mybir as mybir
from concourse.bass_utils import run_bass_kernel_spmd

# ---------------------------------------------------------------- constants
FULL_SHAPE = (64, 4096, 1024)
N_CORES = 8
DT, P, FD = 32, 128, 8192  # per-core: 32 DMA tiles of [128, 8192]
FC = FD // 2               # compute half-tile free dim
H = 2 * DT                 # number of compute half-tiles

N_SL = 4   # DMA slot ring depth (in and out)
N_C = 3    # compute ring depth (kq / ys / vt), in half-tiles

# k is shifted down by an integer constant so the DVE intermediate
# vt = x' - (k-11) stays in [0.7, 1.7] where fp16 has ~2^-11 ulp
# (integer shifts commute with RNE quantization, so semantics are unchanged).
KQ_SHIFT = 11

RHO = math.exp(0.875) - 1.0
CONST = 1.0 + RHO / 32768.0          # +0.5 LSB rounding offset of t_fx in Q14
B_SL = RHO / CONST                   # k-coefficient before unit-rescale
AK_SCALE = 8.0 / 7.0                 # 65536/57344
AK_BIAS = 153.0 / 14.0               # 655360/57344 - 0.5
A2_SCALE = 0.875
A2_BIAS = -10.0 + math.log(CONST) + math.log(B_SL)
CONST1 = 1.0 + (655360.0 / 57344.0) * RHO / CONST
T3_ADD = CONST1 / B_SL
# shifted-k variants
AK_BIAS_S = AK_BIAS - KQ_SHIFT
A2_BIAS_S = A2_BIAS + A2_SCALE * KQ_SHIFT
T3_ADD_S = T3_ADD - KQ_SHIFT         # host prescale offset: x' = (8/7)x + T3_ADD_S
QBIAS = AK_BIAS_S - T3_ADD_S         # quantizer bias applied to x'

# host-fixup region boundaries (float32 compares on raw x)
FIX_HI = np.float32(2.7773)          # below first int32-wrap threshold (2.77735)
FIX_LO = np.float32(-4.7)            # deep tail: LUT quantization breaks the model

# ------------------------------------------------------------ bass builder
_NC = None


def _quant_on_act(h: int) -> bool:
    # 5/12 of half-tiles quantize on ScalarE (Copy activation, 3.7us), the
    # rest on DVE (tensor_scalar, ~1.3us) — balances both engines at ~340us
    return h % 12 >= 7


def _build_nc() -> bass.Bass:
    global _NC
    if _NC is not None:
        return _NC
    f32, f16, i16 = mybir.dt.float32, mybir.dt.float16, mybir.dt.int16
    nc = bass.Bass()
    x_ext = nc.declare_dram_parameter("x", [DT, P, FD], f16, isOutput=False)
    o_ext = nc.declare_dram_parameter("out", [DT, P, FD], f16, isOutput=True)

    # [128,1] constant for the Exp activation bias (const_aps only has 0/1).
    bias_t = nc.alloc_sbuf_tensor("const-a2bias", [P, 1], f32)
    a2_bias_ap = bias_t.ap()

    ctx = ExitStack()
    # One backing tensor per DMA ring so each 8192-wide DMA slot is two
    # contiguous 4096-wide compute halves.
    xt = ctx.enter_context(nc.sbuf_tensor("xt", [P, N_SL * FD], f16))
    ot = ctx.enter_context(nc.sbuf_tensor("ot", [P, N_SL * FD], f16))
    kq = [ctx.enter_context(nc.sbuf_tensor(f"kq{j}", [P, FC], i16)) for j in range(N_C)]
    ys = [ctx.enter_context(nc.sbuf_tensor(f"ys{j}", [P, FC], f16)) for j in range(N_C)]
    vt = [ctx.enter_context(nc.sbuf_tensor(f"vt{j}", [P, FC], f16)) for j in range(N_C)]
    s_in = [ctx.enter_context(nc.semaphore(f"s_in{j}")) for j in range(N_SL)]
    s_out = [ctx.enter_context(nc.semaphore(f"s_out{j}")) for j in range(N_SL)]
    s_ka = ctx.enter_context(nc.semaphore("s_ka"))  # ACT-produced quants
    s_kd = ctx.enter_context(nc.semaphore("s_kd"))  # DVE-produced quants
    s_y = ctx.enter_context(nc.semaphore("s_y"))
    s_v1 = ctx.enter_context(nc.semaphore("s_v1"))
    s_o = ctx.enter_context(nc.semaphore("s_o"))
    s_bias = ctx.enter_context(nc.semaphore("s_bias"))
    block = ctx.enter_context(nc.Block())

    # cumulative quant counts: after quant(h), s_ka == NA[h], s_kg == NG[h]
    NA, NG = [], []
    na = ng = 0
    for h in range(H):
        if _quant_on_act(h):
            na += 1
        else:
            ng += 1
        NA.append(na)
        NG.append(ng)

    def wait_quant(engine, h):
        """Wait until quant(h) is complete (from either producer)."""
        if _quant_on_act(h):
            engine.wait_ge(s_ka, NA[h])
        else:
            engine.wait_ge(s_kd, NG[h])

    def xh(h):  # compute half h of the input ring
        t = (h // 2) % N_SL
        return xt[:, t * FD + (h % 2) * FC : t * FD + (h % 2) * FC + FC]

    def oh(h):  # compute half h of the output ring
        t = (h // 2) % N_SL
        return ot[:, t * FD + (h % 2) * FC : t * FD + (h % 2) * FC + FC]

    def in_wait(engine, h):
        t = h // 2
        engine.wait_ge(s_in[t % N_SL], 16 * (t // N_SL + 1))

    @block.sync
    def _(sync):
        for t in range(DT):
            if t >= N_SL:
                # slot (t-N_SL) free once both halves went through DVE T2
                # (T2 consumed xt and kq, so the quant read is implied)
                sync.wait_ge(s_v1, 2 * (t - N_SL) + 2)
            sync.dma_start(
                out=xt[:, (t % N_SL) * FD : (t % N_SL + 1) * FD], in_=x_ext[t]
            ).then_inc(s_in[t % N_SL], 16)

    @block.gpsimd
    def _(gpsimd):
        nc.gpsimd.memset(a2_bias_ap, A2_BIAS_S).then_inc(s_bias, 1)

    @block.scalar
    def _(scalar):
        scalar.wait_ge(s_bias, 1)
        for h in range(H):
            t = h // 2
            if _quant_on_act(h):
                in_wait(scalar, h)
                if h >= N_C:
                    scalar.wait_ge(s_v1, h - N_C + 1)  # kq slot: T2(h-N_C)
                nc.scalar.activation(
                    kq[h % N_C][:], xh(h), mybir.ActivationFunctionType.Copy,
                    bias=QBIAS, scale=1.0,
                ).then_inc(s_ka, 1)
            if h >= N_C:
                scalar.wait_ge(s_o, h - N_C + 1)  # ys slot: T3(h-N_C) done
            wait_quant(scalar, h)  # covers own-engine ACT pipeline RAW too
            nc.scalar.activation(
                ys[h % N_C][:], kq[h % N_C][:], mybir.ActivationFunctionType.Exp,
                bias=a2_bias_ap, scale=A2_SCALE,
            ).then_inc(s_y, 1)
            # Output DMA for tile (h-3)//2, dispatched well after its T3 so
            # this wait is almost always already satisfied.
            if h % 2 == 1 and h >= 3:
                to = (h - 3) // 2
                scalar.wait_ge(s_o, 2 * to + 2)
                nc.scalar.dma_start(
                    out=o_ext[to], in_=ot[:, (to % N_SL) * FD : (to % N_SL + 1) * FD]
                ).then_inc(s_out[to % N_SL], 16)
        for to in range(DT - 1, DT):  # flush the last tile
            scalar.wait_ge(s_o, 2 * to + 2)
            nc.scalar.dma_start(
                out=o_ext[to], in_=ot[:, (to % N_SL) * FD : (to % N_SL + 1) * FD]
            ).then_inc(s_out[to % N_SL], 16)

    @block.vector
    def _(vector):
        # Software-pipelined by one half-tile: quant/T2 for h+1 issue before
        # T3(h), so the ACT Exp(h) latency hides behind T2(h+1).
        def front(h):
            in_wait(vector, h)
            if _quant_on_act(h):
                wait_quant(vector, h)
            else:
                if h >= N_C:
                    vector.wait_ge(s_y, h - N_C + 1)  # kq slot: Exp(h-N_C) done
                nc.vector.tensor_scalar(
                    out=kq[h % N_C][:], in0=xh(h), scalar1=QBIAS, scalar2=None,
                    op0=mybir.AluOpType.add,
                ).then_inc(s_kd, 1)
                vector.wait_ge(s_kd, NG[h])  # own-engine RAW on kq
            if h >= N_C:
                vector.wait_ge(s_o, h - N_C + 1)  # vt slot: T3(h-N_C) done (own)
            # T2: vt = x' - kq   (fp16/int16 -> 2x)
            nc.vector.tensor_tensor(
                out=vt[h % N_C][:], in0=xh(h), in1=kq[h % N_C][:],
                op=mybir.AluOpType.subtract,
            ).then_inc(s_v1, 1)

        def back(h):
            t = h // 2
            vector.wait_ge(s_y, h + 1)
            vector.wait_ge(s_v1, h + 1)  # own-engine RAW on vt (pipeline drain)
            if t >= N_SL and h % 2 == 0:
                vector.wait_ge(s_out[t % N_SL], 16 * (t // N_SL))  # slot drained
            # T3: ot = vt * ys   (all operands fp16 -> 2x)
            nc.vector.tensor_tensor(
                out=oh(h), in0=vt[h % N_C][:], in1=ys[h % N_C][:],
                op=mybir.AluOpType.mult,
            ).then_inc(s_o, 1)

        front(0)
        for h in range(1, H):
            front(h)
            back(h - 1)
        back(H - 1)

    ctx.close()
    _NC = nc
    return nc


# ------------------------------------------------- exact host-side reference
_XP = np.round(np.linspace(-10.0, 4.0, 17) * 65536.0).astype(np.int64)
_YV = np.round(np.exp(np.linspace(-10.0, 4.0, 17)) * 16384.0).astype(np.int64)
_DY = np.diff(_YV)


def _reference_exact(xs: np.ndarray) -> np.ndarray:
    """Bit-faithful int32 reference for a (small) subset of elements."""
    x_int = np.rint(xs.astype(np.float64) * 65536.0).astype(np.int64)
    mask_low = x_int <= _XP[0]
    mask_high = x_int >= _XP[-1]
    xc = np.clip(x_int, _XP[0], _XP[-1])
    idx = np.clip(np.searchsorted(_XP, xc, side="left") - 1, 0, 15)
    dxv = xc - _XP[idx]
    t_fx = ((dxv << 14) + 28672) // 57344
    prod = t_fx * _DY[idx] + 8192
    pm = prod & 0xFFFFFFFF
    S = np.where(pm >= 1 << 31, pm - (1 << 32), pm)
    interp = _YV[idx] + (S >> 14)
    out_int = np.where(mask_low, _YV[0], np.where(mask_high, _YV[-1], interp))
    return (out_int.astype(np.float32) / np.float32(16384.0)).astype(np.float32)


def _host_fixup(x_flat: np.ndarray, out_flat: np.ndarray) -> None:
    sel = (x_flat >= FIX_HI) | (x_flat < FIX_LO)
    idxs = np.flatnonzero(sel)
    if idxs.size:
        out_flat[idxs] = _reference_exact(x_flat[idxs])


_last_results = None


def kernel(x: np.ndarray) -> np.ndarray:
    assert x.shape == FULL_SHAPE and x.dtype == np.float32, (x.shape, x.dtype)
    nc = _build_nc()
    per = FULL_SHAPE[0] // N_CORES
    # host prescale to fp16
    xp = (np.float32(AK_SCALE) * x + np.float32(T3_ADD_S)).astype(np.float16)
    in_maps = [
        {"x": np.ascontiguousarray(xp[i * per : (i + 1) * per]).reshape(DT, P, FD)}
        for i in range(N_CORES)
    ]
    global _last_results
    res = run_bass_kernel_spmd(nc, in_maps, core_ids=list(range(N_CORES)))
    _last_results = res
    out = np.concatenate(
        [
            r["out"].astype(np.float32).reshape(per, FULL_SHAPE[1], FULL_SHAPE[2])
            for r in res.results
        ],
        axis=0,
    )
    _host_fixup(x.ravel(), out.ravel())
    return out
